# revision 8
# baseline (speedup 1.0000x reference)
"""ArcNegFace loss kernel for 8 TRN2 NeuronCores.

Strategy: model-parallel classification head. The weight matrix
[100000, 512] is sharded over its out_features axis across the 8 cores
(padded to 102400 rows -> 12800 rows / core = 25 uniform column-tiles of
512). Each core computes its [256, 12800] slice of the logits.

The label-gather is done host-side (gather of 256 weight rows,
replicated to every core); each core recomputes cos_lb / a_lb in f32
locally (tiny), so no collective is needed. The one-hot "positive"
logits (256 scalars) are patched host-side from a device-computed a_lb
output during unsharding.

Per-core dataflow (all 25 column tiles, software-pipelined by Tile):
  HBM --SWDGE cast f32->fp16--> w_nat [128c, 4, 512d]
  ssq_c   = sum_d w^2        (fused Square+accum on ACT / TTR on DVE)
  rnorm   = exp(-0.5*ln(ssq)) (stays inside one ACT table set)
  wn      = w * rnorm         (per-partition tensor_scalar, fp16 4x)
  wT      = xbar DMA-transpose (fp16, SBUF->SBUF)
  cos     = xnT.T @ wT        (PE, K=512 accumulated in PSUM, fp16)
  d2      = Square(cos - a)   (ACT, PSUM src, per-partition bias)
  f       = Exp(-d2/sigma + ln(SCALE*ALPHA))   (ACT)
  s       = (cos + 1) * f     (DVE scalar_tensor_tensor, PSUM src)
  o       = s - SCALE         (DVE tensor_scalar, fp16 out)
  HBM <-- o (fp16; host casts to f32)
"""

import math

import numpy as np

B, D, C = 256, 512, 100000
NCORES = 8
CSH = 12800                 # padded columns per core
CPAD = CSH * NCORES         # 102400
CTILE = 512
NT = CSH // CTILE           # 25
SCALE = 64.0
MARGIN = 0.5
ALPHA = 1.2
SIGMA = 2.0
THRESH = math.cos(math.pi - MARGIN)
MM_ = math.sin(math.pi - MARGIN) * MARGIN
COS_M = math.cos(MARGIN)
SIN_M = math.sin(MARGIN)
K1 = SCALE * ALPHA
LNK1 = math.log(K1)

# Of every 20 ssq column-slices, this many run on DVE (tensor_tensor_reduce);
# the rest run on ACT (Square + accum_out). Balances the two engines.
SSQ_DVE_OF_20 = 9

_CACHE: dict = {}


def _build():
    from contextlib import ExitStack

    import concourse.bacc as bacc
    import concourse.tile as tile
    from concourse import mybir

    f32 = mybir.dt.float32
    f16 = mybir.dt.float16
    Alu = mybir.AluOpType
    Act = mybir.ActivationFunctionType

    nc = bacc.Bacc(
        "TRN2", target_bir_lowering=False, debug=False, num_devices=NCORES
    )
    inp_e = nc.dram_tensor("inp", [B, D], f32, kind="ExternalInput").ap()
    wlab_e = nc.dram_tensor("wlab", [B, D], f32, kind="ExternalInput").ap()
    w_e = nc.dram_tensor("w", [CSH, D], f32, kind="ExternalInput").ap()
    out_e = nc.dram_tensor("out", [B, CSH], f16, kind="ExternalOutput").ap()
    alb_e = nc.dram_tensor("alb", [128, 2], f32, kind="ExternalOutput").ap()

    with tile.TileContext(nc) as tc, ExitStack() as ctx:
        singles = ctx.enter_context(tc.tile_pool(name="singles", bufs=1))
        wpool = ctx.enter_context(tc.tile_pool(name="wpool", bufs=3))
        wtpool = ctx.enter_context(tc.tile_pool(name="wtpool", bufs=3))
        spool = ctx.enter_context(tc.tile_pool(name="spool", bufs=4))
        tpool = ctx.enter_context(tc.tile_pool(name="tpool", bufs=3))
        epool = ctx.enter_context(tc.tile_pool(name="epool", bufs=3))
        opool = ctx.enter_context(tc.tile_pool(name="opool", bufs=3))
        psum = ctx.enter_context(tc.tile_pool(name="psum", bufs=4, space="PSUM"))

        # ---------------- x / wlab prep (tiny, one-time) ----------------
        xt = singles.tile([128, 2, D], f32)
        nc.gpsimd.dma_start(xt, inp_e.rearrange("(j p) d -> p j d", p=128))
        wl = singles.tile([128, 2, D], f32)
        nc.gpsimd.dma_start(wl, wlab_e.rearrange("(j p) d -> p j d", p=128))

        ssqx = singles.tile([128, 2], f32)
        ssql = singles.tile([128, 2], f32)
        for j in range(2):
            tr = tpool.tile([128, D], f32, tag="preptrash")
            nc.scalar.activation(tr, xt[:, j], Act.Square,
                                 accum_out=ssqx[:, j:j + 1])
            tr = tpool.tile([128, D], f32, tag="preptrash")
            nc.scalar.activation(tr, wl[:, j], Act.Square,
                                 accum_out=ssql[:, j:j + 1])

        # rnorm = exp(-0.5*ln(ssq))  (avoids Sqrt -> no ACT table switch)
        lnx = singles.tile([128, 2], f32)
        nc.scalar.activation(lnx, ssqx, Act.Ln)
        rnx = singles.tile([128, 2], f32)
        nc.scalar.activation(rnx, lnx, Act.Exp, scale=-0.5)
        lnl = singles.tile([128, 2], f32)
        nc.scalar.activation(lnl, ssql, Act.Ln)
        rnl = singles.tile([128, 2], f32)
        nc.scalar.activation(rnl, lnl, Act.Exp, scale=-0.5)

        xn16 = singles.tile([128, 2, D], f16)
        xnf = singles.tile([128, 2, D], f32)
        wlf = singles.tile([128, 2, D], f32)
        for j in range(2):
            nc.vector.tensor_scalar(xn16[:, j], xt[:, j], rnx[:, j:j + 1],
                                    None, Alu.mult)
            nc.vector.tensor_scalar(xnf[:, j], xt[:, j], rnx[:, j:j + 1],
                                    None, Alu.mult)
            nc.vector.tensor_scalar(wlf[:, j], wl[:, j], rnl[:, j:j + 1],
                                    None, Alu.mult)

        # cos_lb[b] = xn[b] . wn_label[b]   (f32)
        coslb = singles.tile([128, 2], f32)
        for j in range(2):
            tr = tpool.tile([128, D], f32, tag="preptrash")
            nc.vector.scalar_tensor_tensor(
                tr, xnf[:, j], 1.0, wlf[:, j], Alu.mult, Alu.mult,
                accum_out=coslb[:, j:j + 1])

        # a_lb = cos_lb > THRESH ? cos(acos(clip(cos_lb)) + m) : cos_lb - mm
        #      = c*cos(m) - sin(m)*sqrt(1-c^2)   (branch 1, c clipped)
        cmin = singles.tile([128, 2], f32)
        nc.vector.tensor_scalar(cmin, coslb, 1.0, -1.0, Alu.min, Alu.max)
        csq = singles.tile([128, 2], f32)
        nc.scalar.activation(csq, cmin, Act.Square)
        y1 = singles.tile([128, 2], f32)
        nc.vector.tensor_scalar(y1, csq, -1.0, 1.0, Alu.mult, Alu.add)
        lny = singles.tile([128, 2], f32)
        nc.scalar.activation(lny, y1, Act.Ln)
        sn = singles.tile([128, 2], f32)  # sqrt(1-c^2) = exp(0.5*ln(1-c^2))
        nc.scalar.activation(sn, lny, Act.Exp, scale=0.5)
        b1 = singles.tile([128, 2], f32)
        nc.vector.tensor_scalar(b1, cmin, COS_M, None, Alu.mult)
        snm = singles.tile([128, 2], f32)
        nc.vector.tensor_scalar(snm, sn, -SIN_M, None, Alu.mult)
        nc.vector.tensor_tensor(b1, b1, snm, Alu.add)
        b2 = singles.tile([128, 2], f32)
        nc.vector.tensor_scalar(b2, coslb, MM_, None, Alu.subtract)
        mask = singles.tile([128, 2], mybir.dt.uint8)
        nc.vector.tensor_scalar(mask, coslb, THRESH, None, Alu.is_gt)
        alb = singles.tile([128, 2], f32)
        nc.vector.select(alb, mask, b1, b2)
        nega = singles.tile([128, 2], f32)
        nc.vector.tensor_scalar(nega, alb, -1.0, None, Alu.mult)
        nc.gpsimd.dma_start(alb_e, alb)

        lnk1 = singles.tile([128, 1], f32)
        nc.vector.memset(lnk1, LNK1)

        # xnT[j2][p, k, b] = xn[j2*128 + b, k*128 + p]   (fp16)
        xnT = [singles.tile([128, 4, 128], f16, tag=f"xnT{j}", name=f"xnT{j}")
               for j in range(2)]
        for j in range(2):
            nc.sync.dma_start_transpose(xnT[j], xn16[:, j])

        # ---------------- main loop over 25 column tiles ----------------
        idx = 0
        for t in range(NT):
            wnat = wpool.tile([128, 4, CTILE], f16, tag="wnat")
            nc.gpsimd.dma_start(
                wnat,
                w_e[t * CTILE:(t + 1) * CTILE].rearrange(
                    "(j p) d -> p j d", p=128))

            ssq = spool.tile([128, 4], f32, tag="ssq")
            for j in range(4):
                tr16 = tpool.tile([128, CTILE], f16, tag="trash16")
                if (idx % 20) < SSQ_DVE_OF_20:
                    nc.vector.scalar_tensor_tensor(
                        tr16, wnat[:, j], 1.0, wnat[:, j],
                        Alu.mult, Alu.mult, accum_out=ssq[:, j:j + 1])
                else:
                    nc.scalar.activation(tr16, wnat[:, j], Act.Square,
                                         accum_out=ssq[:, j:j + 1])
                idx += 1

            lnw = spool.tile([128, 4], f32, tag="lnw")
            nc.scalar.activation(lnw, ssq, Act.Ln)
            rnw = spool.tile([128, 4], f32, tag="rnw")
            nc.scalar.activation(rnw, lnw, Act.Exp, scale=-0.5)
            for j in range(4):
                nc.vector.tensor_scalar(wnat[:, j], wnat[:, j],
                                        rnw[:, j:j + 1], None, Alu.mult)

            # wT[p, j, k, c] = wn[j*128 + c, k*128 + p]
            wT = wtpool.tile([128, 4, 4, 128], f16, tag="wT")
            for j in range(4):
                nc.sync.dma_start_transpose(wT[:, j], wnat[:, j])

            for j2 in range(2):
                pc = psum.tile([128, CTILE], f32, tag="pc")
                for k in range(4):
                    nc.tensor.matmul(pc, lhsT=xnT[j2][:, k], rhs=wT[:, :, k],
                                     start=(k == 0), stop=(k == 3))
                d2 = epool.tile([128, CTILE], f32, tag="d2")
                nc.scalar.activation(d2, pc, Act.Square,
                                     bias=nega[:, j2:j2 + 1])
                f_ = epool.tile([128, CTILE], f32, tag="f")
                nc.scalar.activation(f_, d2, Act.Exp, bias=lnk1,
                                     scale=-1.0 / SIGMA)
                s_ = epool.tile([128, CTILE], f32, tag="s")
                nc.vector.scalar_tensor_tensor(s_, pc, 1.0, f_,
                                               Alu.add, Alu.mult)
                o_ = opool.tile([128, CTILE], f16, tag="o")
                nc.vector.tensor_scalar(o_, s_, SCALE, None, Alu.subtract)
                nc.gpsimd.dma_start(
                    out_e[j2 * 128:(j2 + 1) * 128,
                          t * CTILE:(t + 1) * CTILE], o_)

    nc.compile()
    return nc


def _get_nc():
    nc = _CACHE.get("nc")
    if nc is None:
        nc = _build()
        _CACHE["nc"] = nc
    return nc


def _run(in_maps, trace=False, tmpdir=None):
    from concourse.bass_utils import run_bass_kernel_spmd

    nc = _get_nc()
    return run_bass_kernel_spmd(
        nc, in_maps, core_ids=list(range(NCORES)), trace=trace, tmpdir=tmpdir)


def make_in_maps(input, label, weight):
    inp = np.ascontiguousarray(np.asarray(input, dtype=np.float32))
    lab = np.asarray(label).astype(np.int64)
    w = np.ascontiguousarray(np.asarray(weight, dtype=np.float32))
    wlab = np.ascontiguousarray(w[lab])
    wpad = np.concatenate([w, np.ones((CPAD - C, D), np.float32)], axis=0)
    in_maps = [
        {"inp": inp, "wlab": wlab,
         "w": np.ascontiguousarray(wpad[i * CSH:(i + 1) * CSH])}
        for i in range(NCORES)
    ]
    return in_maps, lab


def assemble(results, lab):
    full = np.concatenate(
        [results[i]["out"] for i in range(NCORES)], axis=1
    )[:, :C].astype(np.float32)
    alb = np.asarray(results[0]["alb"], dtype=np.float32)  # [128, 2]
    a_vec = alb.transpose(1, 0).reshape(B)
    full[np.arange(B), lab] = (SCALE * a_vec).astype(np.float32)
    return full


def kernel(input, label, weight):
    in_maps, lab = make_in_maps(input, label, weight)
    res = _run(in_maps)
    return assemble(res.results, lab)
